# revision 1
# baseline (speedup 1.0000x reference)
"""Fused multi-head attention kernel for Trainium2 (8 NeuronCores, SPMD).

Problem: B=512, T=128, C=768, H=12, D=64 causal MHA:
    qkv = x @ w_qkv.T ; per-head causal softmax(q k^T / 8) @ v ; proj + bias.

Sharding: data-parallel over batch, 64 batches per core. Host-side prep is
layout only (transposes); all FLOPs run on device.

Per-core dataflow per batch (T=128 tokens on 128 partitions):
  - q^T,k^T [12*128, t] chunks via f32r matmuls (w stationary, 4-batch
    groups so the moving dim is 512), cast to bf16 on PSUM evacuation.
  - v in natural [t, C] layout via f32r matmuls, evacuated into an
    augmented [t, 12, 65] tile whose 65th column is ones.
  - attention in head quads: S^T = k^T.T @ q^T directly in [s, t] layout
    (4 heads share one PSUM bank), one exp pass (ACT, scale 1/8), causal
    mask applied AFTER exp as affine_select-to-zero on the idle GpSimd,
    O_aug[t, 65] = expST.T @ [v_h | 1] so column 64 carries the softmax
    denominator per partition; one reciprocal + one broadcast-multiply
    normalizes 4 heads at once. Head pairs are PE-transposed into
    proj-ready [c, t] chunks.
  - proj: f32r matmuls (OT chunk stationary), bias added during PSUM
    evacuation, DMA out in natural [t, C] layout.
"""

import numpy as np

import concourse.bass as bass
import concourse.tile as tile
from concourse import bacc, mybir
from concourse.bass_utils import run_bass_kernel_spmd
from concourse.masks import make_identity

F32 = mybir.dt.float32
F32R = mybir.dt.float32r
BF16 = mybir.dt.bfloat16

N_CORES = 8
B_TOTAL = 512
T = 128
C = 768
H = 12
D = 64
KC = C // 128  # 6 contraction chunks
B_CORE = B_TOTAL // N_CORES  # 64
GB = 4  # batches per group (moving dim 4*128=512)


def _build(b_core=B_CORE, att_bf16=True, safe_mask=False, safe_memset=False, safe_norm=False, stage=5):
    nc = bacc.Bacc()
    xT_h = nc.dram_tensor("xT", [b_core, KC, 128, T], BF16, kind="ExternalInput")
    wqkvT_h = nc.dram_tensor("wqkvT", [C, 3 * C], BF16, kind="ExternalInput")
    wpT_h = nc.dram_tensor("wpT", [C, C], F32R, kind="ExternalInput")
    bias_h = nc.dram_tensor("bias", [C], F32, kind="ExternalInput")
    y_h = nc.dram_tensor("y", [b_core, T, C], F32, kind="ExternalOutput")

    att_dt = BF16 if att_bf16 else F32
    n_groups = b_core // GB

    with tile.TileContext(nc) as tc:
        with (
            tc.tile_pool(name="const", bufs=1) as constp,
            tc.tile_pool(name="xt", bufs=2) as xtp,
            tc.tile_pool(name="qkt", bufs=2) as qktp,
            tc.tile_pool(name="vsb", bufs=2) as vp,
            tc.tile_pool(name="ot", bufs=2) as otp,
            tc.tile_pool(name="ysb", bufs=2) as yp,
            tc.tile_pool(name="small", bufs=3) as smallp,
            tc.tile_pool(name="stats", bufs=3) as statsp,
            tc.tile_pool(name="qkps", bufs=2, space="PSUM") as qkpsp,
            tc.tile_pool(name="sqps", bufs=3, space="PSUM") as sqpsp,
            tc.tile_pool(name="oaps", bufs=2, space="PSUM") as oapsp,
            tc.tile_pool(name="pjps", bufs=1, space="PSUM") as pjpsp,
        ):
            # ---- constants / weights (loaded once) ----
            wqkv = constp.tile([128, KC, 3 * C], BF16, tag="wqkv")
            nc.sync.dma_start(
                out=wqkv[:], in_=wqkvT_h[:].rearrange("(k p) o -> p k o", p=128)
            )
            wp = constp.tile([128, KC, C], F32R, tag="wp")
            nc.sync.dma_start(
                out=wp[:], in_=wpT_h[:].rearrange("(k p) o -> p k o", p=128)
            )
            bias_bc = constp.tile([128, C], F32, tag="bias")
            b_src = bias_h[:]
            b_bcast = bass.AP(
                tensor=b_src.tensor, offset=b_src.offset, ap=[[0, 128]] + list(b_src.ap)
            )
            nc.gpsimd.dma_start(out=bias_bc[:], in_=b_bcast)

            ident = constp.tile([128, 128], F32, tag="ident")
            make_identity(nc, ident[:])
            mask01 = constp.tile([128, T], att_dt, tag="mask01")
            nc.gpsimd.memset(mask01[:], 1.0)
            nc.gpsimd.affine_select(
                out=mask01[:],
                in_=mask01[:],
                compare_op=mybir.AluOpType.is_ge,
                fill=0.0,
                base=127,
                pattern=[[1, T]],
                channel_multiplier=0,
            )

            for g in range(n_groups):
                # ---- load 4 batches of xT ----
                xt = xtp.tile([128, KC, GB, T], BF16, tag="xt")
                for bi in range(GB):
                    b = g * GB + bi
                    nc.sync.dma_start(
                        out=xt[:, :, bi, :],
                        in_=xT_h[b].rearrange("k p t -> p k t"),
                    )

                # ---- q^T, k^T chunks for the whole group ----
                qkt = qktp.tile([128, 2 * KC, GB, T], att_dt, tag="qkt")
                for r in range(2 * KC):
                    ps = qkpsp.tile([128, GB, T], F32, tag="qkps")
                    for kc in range(KC):
                        nc.tensor.matmul(
                            ps[:],
                            lhsT=wqkv[:, kc, 128 * r : 128 * r + 128],
                            rhs=xt[:, kc, :, :],
                            start=(kc == 0),
                            stop=(kc == KC - 1),
                        )
                    if r % 2 == 0:
                        nc.vector.tensor_copy(qkt[:, r], ps[:])
                    else:
                        nc.scalar.copy(qkt[:, r], ps[:])

                for bi in range(GB):
                    b = g * GB + bi
                    # ---- v into augmented [t, 12, 65] tile (ones in col 64) --
                    vaug = vp.tile([128, H, D + 1], att_dt, tag="vaug")
                    if safe_memset:
                        nc.gpsimd.memset(vaug[:], 1.0)
                    else:
                        nc.gpsimd.memset(vaug[:, :, D : D + 1], 1.0)
                    for half in range(2):
                        vps = qkpsp.tile([128, 6, D], F32, tag="qkps")
                        for kc in range(KC):
                            nc.tensor.matmul(
                                vps[:],
                                lhsT=xt[:, kc, bi, :],
                                rhs=wqkv[
                                    :, kc, 2 * C + 384 * half : 2 * C + 384 * (half + 1)
                                ],
                                start=(kc == 0),
                                stop=(kc == KC - 1),
                            )
                        nc.scalar.copy(vaug[:, 6 * half : 6 * half + 6, 0:D], vps[:])

                    if stage == 1:
                        ysb = yp.tile([128, C], F32, tag="ysb")
                        nc.vector.tensor_copy(ysb[:].rearrange("p (h d) -> p h d", h=H), vaug[:, :, 0:D])
                        nc.sync.dma_start(out=y_h[b], in_=ysb[:])
                        continue
                    # ---- attention in head quads ----
                    ot = otp.tile([128, KC, T], F32R, tag="ot")
                    for q4 in range(H // 4):
                        expq = smallp.tile([128, 4, T], att_dt, tag="expq")
                        for j in range(4):
                            h = 4 * q4 + j
                            po = 64 * (h % 2)
                            ch = h // 2
                            # S^T[s, t] = sum_d kT[d, s] qT[d, t]
                            sqj = sqpsp.tile([128, T], F32, tag="sqps")
                            nc.tensor.matmul(
                                sqj[:],
                                lhsT=qkt[po : po + 64, KC + ch, bi, :],
                                rhs=qkt[po : po + 64, ch, bi, :],
                                start=True,
                                stop=True,
                            )
                            nc.scalar.activation(
                                out=expq[:, j, :],
                                in_=sqj[:],
                                func=mybir.ActivationFunctionType.Exp,
                                scale=0.125,
                            )
                            # causal: zero out s > t (partition=s, free=t)
                            if safe_mask:
                                nc.vector.tensor_mul(
                                    expq[:, j, :], expq[:, j, :], mask01[:]
                                )
                            else:
                                nc.gpsimd.affine_select(
                                    out=expq[:, j, :],
                                    in_=expq[:, j, :],
                                    compare_op=mybir.AluOpType.is_ge,
                                    fill=0.0,
                                    base=0,
                                    pattern=[[1, T]],
                                    channel_multiplier=-1,
                                )

                        if stage == 2:
                            if q4 == 0:
                                ysb = yp.tile([128, C], F32, tag="ysb")
                            nc.vector.tensor_copy(
                                ysb[:, 256 * q4 : 256 * (q4 + 1)], expq[:, 0:2, :]
                            )
                            if q4 == 2:
                                nc.sync.dma_start(out=y_h[b], in_=ysb[:])
                            continue
                        o4 = smallp.tile([128, 4, D], F32, tag="o4")
                        for j in range(4):
                            h = 4 * q4 + j
                            oaj = oapsp.tile([128, D + 1], F32, tag="oaps")
                            nc.tensor.matmul(
                                oaj[:],
                                lhsT=expq[:, j, :],
                                rhs=vaug[:, h, :],
                                start=True,
                                stop=True,
                            )
                            recip = statsp.tile([128, 1], F32, tag="recip")
                            nc.vector.reciprocal(recip[:], oaj[:, D : D + 1])
                            nc.vector.tensor_scalar_mul(
                                o4[:, j, :], oaj[:, 0:D], recip[:]
                            )
                        if stage == 3:
                            if q4 == 0:
                                ysb = yp.tile([128, C], F32, tag="ysb")
                            nc.vector.tensor_copy(ysb[:, 256 * q4 : 256 * (q4 + 1)], o4[:])
                            if q4 == 2:
                                nc.sync.dma_start(out=y_h[b], in_=ysb[:])
                            continue
                        for pj in range(2):
                            hp = 2 * q4 + pj
                            otps = sqpsp.tile([128, T], F32, tag="sqps")
                            nc.tensor.transpose(
                                otps[:], o4[:, 2 * pj : 2 * pj + 2, :], ident[:]
                            )
                            nc.scalar.copy(ot[:, hp, :], otps[:])

                    if stage <= 3:
                        continue
                    if stage == 4:
                        ysb = yp.tile([128, C], F32, tag="ysb")
                        nc.vector.tensor_copy(
                            ysb[:].rearrange("p (k t) -> p k t", k=KC), ot[:]
                        )
                        nc.sync.dma_start(out=y_h[b], in_=ysb[:])
                        continue
                    # ---- proj + bias ----
                    ysb = yp.tile([128, C], F32, tag="ysb")
                    for half in range(2):
                        pps = pjpsp.tile([128, 384], F32, tag="pjps")
                        for kc in range(KC):
                            nc.tensor.matmul(
                                pps[:],
                                lhsT=ot[:, kc, :],
                                rhs=wp[:, kc, 384 * half : 384 * (half + 1)],
                                start=(kc == 0),
                                stop=(kc == KC - 1),
                            )
                        nc.vector.tensor_add(
                            ysb[:, 384 * half : 384 * (half + 1)],
                            pps[:],
                            bias_bc[:, 384 * half : 384 * (half + 1)],
                        )
                    nc.sync.dma_start(out=y_h[b], in_=ysb[:])

    nc.finalize()
    return nc


_NC_CACHE = {}


SAFE = dict(safe_mask=False, safe_memset=False, safe_norm=False)
STAGE = [5]


def _get_nc(b_core=B_CORE, att_bf16=True):
    key = (b_core, att_bf16, tuple(sorted(SAFE.items())), STAGE[0])
    if key not in _NC_CACHE:
        _NC_CACHE[key] = _build(b_core, att_bf16, stage=STAGE[0], **SAFE)
    return _NC_CACHE[key]


def _prep_inputs(x, w_qkv, w_proj, b_proj, b_core):
    x = np.asarray(x, dtype=np.float32)
    n_cores = x.shape[0] // b_core
    # [B, T, C] -> [B, C, T] -> [B, KC, 128, T]
    import ml_dtypes

    xT = (
        np.ascontiguousarray(x.transpose(0, 2, 1))
        .reshape(x.shape[0], KC, 128, T)
        .astype(ml_dtypes.bfloat16)
    )
    wqkvT = np.ascontiguousarray(np.asarray(w_qkv, dtype=np.float32).T).astype(
        ml_dtypes.bfloat16
    )
    wpT = np.ascontiguousarray(np.asarray(w_proj, dtype=np.float32).T)
    bias = np.ascontiguousarray(np.asarray(b_proj, dtype=np.float32))
    in_maps = []
    for c in range(n_cores):
        in_maps.append(
            {
                "xT": np.ascontiguousarray(xT[c * b_core : (c + 1) * b_core]),
                "wqkvT": wqkvT,
                "wpT": wpT,
                "bias": bias,
            }
        )
    return in_maps


def run(x, w_qkv, w_proj, b_proj, b_core=B_CORE, att_bf16=True, trace=False):
    nc = _get_nc(b_core, att_bf16)
    n_cores = x.shape[0] // b_core
    in_maps = _prep_inputs(x, w_qkv, w_proj, b_proj, b_core)
    res = run_bass_kernel_spmd(nc, in_maps, list(range(n_cores)), trace=trace)
    y = np.concatenate([res.results[i]["y"] for i in range(n_cores)], axis=0)
    return y, res


def kernel(x, w_qkv, w_proj, b_proj):
    y, _ = run(x, w_qkv, w_proj, b_proj)
    return y



# revision 26
# speedup vs baseline: 1.3393x; 1.3393x over previous
"""Fused multi-head attention kernel for Trainium2 (8 NeuronCores, SPMD).

Problem: B=512, T=128, C=768, H=12, D=64 causal MHA:
    qkv = x @ w_qkv.T ; per-head causal softmax(q k^T / 8) @ v ; proj + bias.

Sharding: data-parallel over batch, 64 batches per core. Host-side prep is
layout only (transposes + bf16 casts); all FLOPs run on device.

Design notes (from HW trace analysis of the previous version):
  - The PE runs long GEMM matmuls (384-512 moving rows) at full rate with
    LDWEIGHTS hidden, but small attention matmuls (65-128 moving rows) are
    dominated by per-instruction LDWEIGHTS + issue overhead when emitted
    back-to-back, and cross-engine latency (exp/mask/normalize) stalls the
    in-order PE queue.
  - So the per-batch program is software-pipelined at instruction
    granularity: the 30 small attention matmuls of batch b are interleaved
    (Bresenham merge) with 42 long GEMM matmuls that have no dependency on
    batch b's attention: proj(b-1), v(b+1), and qkv-chunks of group g+1.
    The small matmuls' weight loads hide under the long matmuls' execution,
    and every cross-engine producer (exp on ACT, mask on DVE, normalize on
    DVE) gets ~1-3us of PE slack before the PE consumes its output.
  - Everything on the PE is bf16 (proj weights/activations included;
    rel-err budget 2e-2 allows it); transposes are bf16 (1 cycle/row).
  - Attention works in 4-head quads: S^T quad -> one exp (ACT, scale 1/8)
    -> one causal mask multiply (DVE) -> per-head O_aug matmuls whose 65th
    column carries the softmax denominator -> reciprocal + 4 scalar-mul
    normalizes -> head-pair PE transposes into proj-ready [c, t] chunks.
  - PSUM budget is exactly 8 banks: qk group GEMM 2, v 1, attention
    (S^T/O_aug/transpose shared tag) 3, proj 2.
"""

import itertools

import numpy as np

import concourse.bass as bass
import concourse.tile as tile
from concourse import bacc, mybir
from concourse.bass_utils import run_bass_kernel_spmd
from concourse.masks import make_identity

F32 = mybir.dt.float32
BF16 = mybir.dt.bfloat16

N_CORES = 8
B_TOTAL = 512
T = 128
C = 768
H = 12
D = 64
KC = C // 128  # 6 contraction chunks
B_CORE = B_TOTAL // N_CORES  # 64
GB = 4  # batches per group (qk moving dim 4*128=512)
N_GROUPS = B_CORE // GB
QK_R = 2 * KC  # 12 r-chunks of 128 rows covering q and k outputs
# qk r-chunks of group g+1 emitted per batch-in-group: 4+4+2+2 = 12,
# front-loaded so the last evacs land well before the next group's S^T
QK_SPLIT = (4, 4, 2, 2)


def _build(b_core=B_CORE):
    nc = bacc.Bacc()
    # xT[b, p, kc, t] = x[b, t, 128*kc + p]
    xT_h = nc.dram_tensor("xT", [b_core, 128, KC, T], BF16, kind="ExternalInput")
    wqkvT_h = nc.dram_tensor("wqkvT", [C, 3 * C], BF16, kind="ExternalInput")
    wpT_h = nc.dram_tensor("wpT", [C, C], BF16, kind="ExternalInput")
    bias_h = nc.dram_tensor("bias", [C], F32, kind="ExternalInput")
    y_h = nc.dram_tensor("y", [b_core, T, C], F32, kind="ExternalOutput")

    n_groups = b_core // GB

    with tile.TileContext(nc) as tc:
        with (
            tc.tile_pool(name="const", bufs=1) as constp,
            tc.tile_pool(name="xt", bufs=2) as xtp,
            tc.tile_pool(name="qkt", bufs=2) as qktp,
            tc.tile_pool(name="vsb", bufs=3) as vp,
            tc.tile_pool(name="expq", bufs=6) as expp,
            tc.tile_pool(name="o4", bufs=3) as o4p,
            tc.tile_pool(name="stats", bufs=3) as statsp,
            tc.tile_pool(name="ot", bufs=2) as otp,
            tc.tile_pool(name="ysb", bufs=2) as yp,
            tc.tile_pool(name="qkps", bufs=2, space="PSUM") as qkpsp,
            tc.tile_pool(name="attps", bufs=4, space="PSUM") as attpsp,
            tc.tile_pool(name="pjps", bufs=2, space="PSUM") as pjpsp,
        ):
            # ---- constants / weights (loaded once) ----
            # wqkv is DMA'd per-kc-chunk so the prologue GEMM can start as
            # soon as its first contraction chunk lands (subtile deps track
            # the per-chunk writes); xt(0) is issued first for the same
            # reason (see prologue below).
            wqkv = constp.tile([128, KC, 3 * C], BF16, tag="wqkv")

            def dma_wqkv():
                for kc in range(KC):
                    nc.sync.dma_start(
                        out=wqkv[:, kc, :],
                        in_=wqkvT_h[128 * kc : 128 * (kc + 1), :],
                    )

            wp = constp.tile([128, KC, C], BF16, tag="wp")

            def dma_wp():
                nc.sync.dma_start(
                    out=wp[:], in_=wpT_h[:].rearrange("(k p) o -> p k o", p=128)
                )

            bias_bc = constp.tile([128, C], F32, tag="bias")
            b_src = bias_h[:]
            b_bcast = bass.AP(
                tensor=b_src.tensor, offset=b_src.offset, ap=[[0, 128]] + list(b_src.ap)
            )
            nc.gpsimd.dma_start(out=bias_bc[:], in_=b_bcast)

            ident = constp.tile([128, 128], BF16, tag="ident")
            make_identity(nc, ident[:])
            # mask4[s, j, t] = 1 if t >= s else 0 (causal keep-mask, 4 heads wide)
            mask4 = constp.tile([128, 4, T], BF16, tag="mask4")
            nc.gpsimd.memset(mask4[:], 1.0)
            for j in range(4):
                nc.gpsimd.affine_select(
                    out=mask4[:, j, :],
                    in_=mask4[:, j, :],
                    compare_op=mybir.AluOpType.is_ge,
                    fill=0.0,
                    base=0,
                    pattern=[[1, T]],
                    channel_multiplier=-1,
                )

            # Per-group / per-batch tile handles, created at emission time.
            xt_sb = {}
            qkt_sb = {}
            vaug_sb = {}
            expq_sb = {}
            ot_sb = {}
            qkt_evac_cnt = [0]

            def dma_xt(g):
                xt = xtp.tile([128, KC, GB, T], BF16, tag="xt", name=f"xt{g}")
                xt_sb[g] = xt
                for bi in range(GB):
                    nc.sync.dma_start(out=xt[:, :, bi, :], in_=xT_h[g * GB + bi])

            def emit_qk_chunk(g, r):
                """One 128-row output chunk of the q/k group GEMM: 6 matmuls."""
                if r == 0:
                    qkt_sb[g] = qktp.tile(
                        [128, QK_R, GB, T], BF16, tag="qkt", name=f"qkt{g}"
                    )
                xt = xt_sb[g]
                ps = qkpsp.tile([128, GB, T], F32, tag="qkps", name=f"qkps{g}_{r}")
                for kc in range(KC):
                    yield nc.tensor.matmul(
                        ps[:],
                        lhsT=wqkv[:, kc, 128 * r : 128 * r + 128],
                        rhs=xt[:, kc, :, :],
                        start=(kc == 0),
                        stop=(kc == KC - 1),
                    )
                # alternate evac engine (GpSimd cannot read PSUM)
                if qkt_evac_cnt[0] % 2 == 0:
                    nc.scalar.copy(qkt_sb[g][:, r, :, :], ps[:])
                else:
                    nc.vector.tensor_copy(qkt_sb[g][:, r, :, :], ps[:])
                qkt_evac_cnt[0] += 1

            def emit_v(b):
                """v for batch b into augmented [t, 12, 65] tile (ones col 64)."""
                g, bi = b // GB, b % GB
                vaug = vp.tile([128, H, D + 1], BF16, tag="vaug", name=f"vaug{b}")
                vaug_sb[b] = vaug
                nc.gpsimd.memset(vaug[:, :, D : D + 1], 1.0)
                xt = xt_sb[g]
                for half in range(2):
                    vps = qkpsp.tile(
                        [128, 6, D], F32, tag="qkps", name=f"vps{b}_{half}"
                    )
                    for kc in range(KC):
                        yield nc.tensor.matmul(
                            vps[:],
                            lhsT=xt[:, kc, bi, :],
                            rhs=wqkv[
                                :, kc, 2 * C + 384 * half : 2 * C + 384 * (half + 1)
                            ],
                            start=(kc == 0),
                            stop=(kc == KC - 1),
                        )
                    nc.scalar.copy(vaug[:, 6 * half : 6 * half + 6, 0:D], vps[:])

            def emit_proj(b):
                """proj + bias for batch b (reads ot_sb[b]), then DMA out."""
                ot = ot_sb.pop(b)
                ysb = yp.tile([128, C], F32, tag="ysb", name=f"ysb{b}")
                pj = [None, None]
                for half in range(2):
                    pj[half] = pjpsp.tile(
                        [128, 384], F32, tag="pjps", name=f"pjps{b}_{half}"
                    )
                for kc in range(KC):
                    for half in range(2):
                        yield nc.tensor.matmul(
                            pj[half][:],
                            lhsT=ot[:, kc, :],
                            rhs=wp[:, kc, 384 * half : 384 * (half + 1)],
                            start=(kc == 0),
                            stop=(kc == KC - 1),
                        )
                for half in range(2):
                    nc.vector.tensor_add(
                        ysb[:, 384 * half : 384 * (half + 1)],
                        pj[half][:],
                        bias_bc[:, 384 * half : 384 * (half + 1)],
                    )
                nc.sync.dma_start(out=y_h[b], in_=ysb[:])

            # Head-slot order within a quad: [4q, 4q+2, 4q+1, 4q+3]. The two
            # even heads use qkt partitions 0:64 (po=0), so their S^T
            # matmuls may share one PSUM tile (sliced writes are legal at
            # po=0) and one exp covers both; the odd heads (po=64) must
            # each write a full PSUM tile at offset 0 (HW quirk). The proj
            # weight rows are permuted host-side to match this slot order
            # (see _prep_inputs).
            QUAD_SLOTS = ((0, 0), (1, 2), (2, 1), (3, 3))  # (slot, head_off)

            def emit_sx(b):
                """S^T + exp + causal mask for batch b (12 PE matmuls)."""
                g, bi = b // GB, b % GB
                qkt = qkt_sb[g]
                expq_l = []
                expq_sb[b] = expq_l
                for q4 in range(3):
                    expq = expp.tile(
                        [128, 4, T], BF16, tag="expq", name=f"expq{b}_{q4}"
                    )
                    expq_l.append(expq)
                    sqp = attpsp.tile(
                        [128, 2, T], F32, tag="att", name=f"sqp{b}_{q4}"
                    )
                    for i, h_off in enumerate((0, 2)):
                        ch = (4 * q4 + h_off) // 2
                        yield nc.tensor.matmul(
                            sqp[:, i, :],
                            lhsT=qkt[0:64, KC + ch, bi, :],
                            rhs=qkt[0:64, ch, bi, :],
                            start=True,
                            stop=True,
                        )
                    nc.scalar.activation(
                        out=expq[:, 0:2, :],
                        in_=sqp[:],
                        func=mybir.ActivationFunctionType.Exp,
                        scale=0.125,
                    )
                    for slot, h_off in ((2, 1), (3, 3)):
                        ch = (4 * q4 + h_off) // 2
                        sqj = attpsp.tile(
                            [128, T], F32, tag="att", name=f"sq{b}_{q4}_{slot}"
                        )
                        yield nc.tensor.matmul(
                            sqj[:],
                            lhsT=qkt[64:128, KC + ch, bi, :],
                            rhs=qkt[64:128, ch, bi, :],
                            start=True,
                            stop=True,
                        )
                        nc.scalar.activation(
                            out=expq[:, slot, :],
                            in_=sqj[:],
                            func=mybir.ActivationFunctionType.Exp,
                            scale=0.125,
                        )
                    nc.gpsimd.tensor_mul(expq[:], expq[:], mask4[:])

            def emit_ot(b):
                """O_aug + normalize + transposes for batch b (18 PE
                matmuls); runs one wall-batch after emit_sx(b)."""
                vaug = vaug_sb.pop(b)
                expq_l = expq_sb.pop(b)
                o4 = [None] * 3
                ot = otp.tile([128, KC, T], BF16, tag="ot", name=f"ot{b}")
                ot_sb[b] = ot
                for q4 in range(3):
                    # per-slot stride padded to 72 f32 (288 B) to keep each
                    # matmul's PSUM write base 16B-aligned
                    oa = attpsp.tile(
                        [128, 4, 72], F32, tag="att", name=f"oa{b}_{q4}"
                    )
                    for slot, h_off in QUAD_SLOTS:
                        yield nc.tensor.matmul(
                            oa[:, slot, 0 : D + 1],
                            lhsT=expq_l[q4][:, slot, :],
                            rhs=vaug[:, 4 * q4 + h_off, :],
                            start=True,
                            stop=True,
                        )
                    recip = statsp.tile(
                        [128, 4, 1], F32, tag="recip", name=f"recip{b}_{q4}"
                    )
                    nc.vector.reciprocal(recip[:], oa[:, :, D : D + 1])
                    o4[q4] = o4p.tile([128, 4, D], BF16, tag="o4", name=f"o4{b}_{q4}")
                    # one broadcast multiply normalizes the whole quad:
                    # recip [128, 4, 1] read with stride-0 over d
                    rsrc = recip[:]
                    rbc = bass.AP(
                        tensor=rsrc.tensor,
                        offset=rsrc.offset,
                        ap=[rsrc.ap[0], rsrc.ap[1], [0, D]],
                    )
                    nc.vector.tensor_mul(o4[q4][:], oa[:, :, 0:D], rbc)
                # slot-pair transposes into proj-ready [c, t] chunks (the
                # host-side wp row permutation matches this c ordering)
                for p in range(KC):
                    q4, pj_ = p // 2, p % 2
                    otps = attpsp.tile([128, T], BF16, tag="att", name=f"otps{b}_{p}")
                    yield nc.tensor.transpose(
                        otps[:], o4[q4][:, 2 * pj_ : 2 * pj_ + 2, :], ident[:]
                    )
                    nc.vector.tensor_copy(ot[:, p, :], otps[:])

            # ---------------- prologue ----------------
            dma_xt(0)
            dma_wqkv()
            if n_groups > 1:
                dma_xt(1)
            dma_wp()
            for r in range(QK_R):
                for _ in emit_qk_chunk(0, r):
                    pass
            for _ in emit_v(0):
                pass

            # ---------------- main loop ----------------
            # 3-deep software pipeline: wall-iteration b emits S^T(b),
            # O/transpose(b-1), proj(b-2), plus v(b+1) and qk chunks of
            # group g+1 as long fillers.
            def merge(att_gens, n_att, fill_gens, n_fill):
                att_iter = itertools.chain(*att_gens)
                fill_iter = itertools.chain(*fill_gens)
                ia = if_ = 0
                while ia < n_att or if_ < n_fill:
                    if if_ < n_fill and (ia >= n_att or if_ * n_att <= ia * n_fill):
                        next(fill_iter, None)
                        if_ += 1
                    else:
                        next(att_iter, None)
                        ia += 1
                for _ in att_iter:
                    pass
                for _ in fill_iter:
                    pass

            for b in range(b_core):
                g, bi = b // GB, b % GB
                # prefetch xt two groups ahead (buffer freed mid-group)
                if bi == 3 and g + 2 < n_groups:
                    dma_xt(g + 2)

                att_gens = []
                n_att = 12
                if b >= 1:
                    att_gens.append(emit_ot(b - 1))
                    n_att += 18
                att_gens.append(emit_sx(b))

                fill_gens = []
                n_fill = 0
                if g + 1 < n_groups:
                    r0 = sum(QK_SPLIT[:bi])
                    for r in range(r0, r0 + QK_SPLIT[bi]):
                        fill_gens.append(emit_qk_chunk(g + 1, r))
                        n_fill += 6
                if b + 1 < b_core:
                    fill_gens.append(emit_v(b + 1))
                    n_fill += 12
                if b >= 2:
                    fill_gens.append(emit_proj(b - 2))
                    n_fill += 12

                merge(att_gens, n_att, fill_gens, n_fill)

            # ---------------- epilogue ----------------
            merge([emit_ot(b_core - 1)], 18, [emit_proj(b_core - 2)], 12)
            for _ in emit_proj(b_core - 1):
                pass

    nc.finalize()
    return nc


_NC_CACHE = {}


def _get_nc(b_core=B_CORE):
    if b_core not in _NC_CACHE:
        _NC_CACHE[b_core] = _build(b_core)
    return _NC_CACHE[b_core]


def _prep_inputs(x, w_qkv, w_proj, b_proj, b_core):
    import ml_dtypes

    x = np.asarray(x, dtype=np.float32)
    n_cores = x.shape[0] // b_core
    # [B, T, C] -> [B, C, T] -> [B, KC, 128, T] -> [B, 128, KC, T]
    xT = (
        np.ascontiguousarray(
            x.transpose(0, 2, 1)
            .reshape(x.shape[0], KC, 128, T)
            .transpose(0, 2, 1, 3)
        )
        .astype(ml_dtypes.bfloat16)
    )
    wqkvT = np.ascontiguousarray(np.asarray(w_qkv, dtype=np.float32).T).astype(
        ml_dtypes.bfloat16
    )
    # The kernel emits attention heads in per-quad slot order
    # [4q, 4q+2, 4q+1, 4q+3], so the proj-input channel order of its ot
    # chunks is permuted; permute the wp rows to match.
    perm = []
    for q4 in range(H // 4):
        for h_off in (0, 2, 1, 3):
            h = 4 * q4 + h_off
            perm.extend(range(D * h, D * h + D))
    wpT = np.ascontiguousarray(
        np.asarray(w_proj, dtype=np.float32).T[perm, :]
    ).astype(ml_dtypes.bfloat16)
    bias = np.ascontiguousarray(np.asarray(b_proj, dtype=np.float32))
    in_maps = []
    for c in range(n_cores):
        in_maps.append(
            {
                "xT": np.ascontiguousarray(xT[c * b_core : (c + 1) * b_core]),
                "wqkvT": wqkvT,
                "wpT": wpT,
                "bias": bias,
            }
        )
    return in_maps


def run(x, w_qkv, w_proj, b_proj, b_core=B_CORE, trace=False):
    nc = _get_nc(b_core)
    n_cores = x.shape[0] // b_core
    in_maps = _prep_inputs(x, w_qkv, w_proj, b_proj, b_core)
    res = run_bass_kernel_spmd(nc, in_maps, list(range(n_cores)), trace=trace)
    y = np.concatenate([res.results[i]["y"] for i in range(n_cores)], axis=0)
    return y, res


def kernel(x, w_qkv, w_proj, b_proj):
    y, _ = run(x, w_qkv, w_proj, b_proj)
    return y


# revision 29
# speedup vs baseline: 1.4661x; 1.0947x over previous
"""Fused multi-head attention kernel for Trainium2 (8 NeuronCores, SPMD).

Problem: B=512, T=128, C=768, H=12, D=64 causal MHA:
    qkv = x @ w_qkv.T ; per-head causal softmax(q k^T / 8) @ v ; proj + bias.

Sharding: data-parallel over batch, 64 batches per core. Host-side prep is
layout only (transposes + bf16 casts); all FLOPs run on device.

Design notes (from HW trace analysis of the f32r baseline at 1042us):
  - The PE runs long GEMM matmuls (384-512 moving rows) at full 2.4GHz rate
    with LDWEIGHTS hidden, but small attention matmuls (65-128 moving rows)
    emitted back-to-back are dominated by per-instruction LDWEIGHTS + issue
    overhead, and cross-engine latency (exp/mask/normalize) stalls the
    in-order PE queue.
  - So the program is a 3-deep software pipeline interleaved at instruction
    granularity: wall-iteration b emits S^T(b), O_aug/transposes(b-1) and
    proj(b-2), with v(b+1) and qkv-chunks of group g+1 as long GEMM
    fillers. Small matmuls are merged in pairs between long fillers
    (Bresenham), so their weight loads hide under GEMM execution and every
    cross-engine producer (exp on ACT, mask on GpSimd, normalize on DVE)
    gets up to a full batch of slack before the PE consumes its output.
  - Everything on the PE is bf16 (proj weights/activations included;
    rel-err budget 2e-2 allows it); transposes are bf16 (1 cycle/row).
  - HW quirk (not modeled by CoreSim): a matmul whose operands sit at SBUF
    partition offset 64 must write PSUM at free-offset 0. Heads are
    therefore processed in per-quad slot order [4q, 4q+2, 4q+1, 4q+3]: the
    two po=0 heads share one sliced PSUM tile + one exp, the two po=64
    heads get full tiles at offset 0; the proj weight rows are permuted
    host-side to match the resulting channel order. GpSimd cannot touch
    PSUM, so it only gets SBUF-only work (causal-mask multiplies).
  - O_aug matmuls append a ones-column of v so column 64 carries the
    softmax denominator; one reciprocal + one stride-0-broadcast multiply
    normalizes 4 heads at once (per-head stride padded to 72 floats to
    keep PSUM write bases 16B-aligned).
  - PSUM budget is exactly 8 banks: qk/v group GEMM 2, attention
    (S^T/O_aug/transpose shared tag) 4, proj 2. Weight DMAs are split
    per-contraction-chunk so the prologue GEMM starts ~5us in.
"""

import itertools

import numpy as np

import concourse.bass as bass
import concourse.tile as tile
from concourse import bacc, mybir
from concourse.bass_utils import run_bass_kernel_spmd
from concourse.masks import make_identity

F32 = mybir.dt.float32
BF16 = mybir.dt.bfloat16

N_CORES = 8
B_TOTAL = 512
T = 128
C = 768
H = 12
D = 64
KC = C // 128  # 6 contraction chunks
B_CORE = B_TOTAL // N_CORES  # 64
GB = 4  # batches per group (qk moving dim 4*128=512)
N_GROUPS = B_CORE // GB
QK_R = 2 * KC  # 12 r-chunks of 128 rows covering q and k outputs
# qk r-chunks of group g+1 emitted per batch-in-group: 4+4+2+2 = 12,
# front-loaded so the last evacs land well before the next group's S^T
QK_SPLIT = (4, 4, 2, 2)


def _build(b_core=B_CORE):
    nc = bacc.Bacc()
    # xT[b, p, kc, t] = x[b, t, 128*kc + p]
    xT_h = nc.dram_tensor("xT", [b_core, 128, KC, T], BF16, kind="ExternalInput")
    wqkvT_h = nc.dram_tensor("wqkvT", [C, 3 * C], BF16, kind="ExternalInput")
    wpT_h = nc.dram_tensor("wpT", [C, C], BF16, kind="ExternalInput")
    bias_h = nc.dram_tensor("bias", [C], F32, kind="ExternalInput")
    y_h = nc.dram_tensor("y", [b_core, T, C], F32, kind="ExternalOutput")

    n_groups = b_core // GB

    with tile.TileContext(nc) as tc:
        with (
            tc.tile_pool(name="const", bufs=1) as constp,
            tc.tile_pool(name="xt", bufs=2) as xtp,
            tc.tile_pool(name="qkt", bufs=2) as qktp,
            tc.tile_pool(name="vsb", bufs=3) as vp,
            tc.tile_pool(name="expq", bufs=6) as expp,
            tc.tile_pool(name="o4", bufs=3) as o4p,
            tc.tile_pool(name="stats", bufs=3) as statsp,
            tc.tile_pool(name="ot", bufs=2) as otp,
            tc.tile_pool(name="ysb", bufs=2) as yp,
            tc.tile_pool(name="qkps", bufs=2, space="PSUM") as qkpsp,
            tc.tile_pool(name="attps", bufs=4, space="PSUM") as attpsp,
            tc.tile_pool(name="pjps", bufs=2, space="PSUM") as pjpsp,
        ):
            # ---- constants / weights (loaded once) ----
            # wqkv is DMA'd per-kc-chunk so the prologue GEMM can start as
            # soon as its first contraction chunk lands (subtile deps track
            # the per-chunk writes); xt(0) is issued first for the same
            # reason (see prologue below).
            wqkv = constp.tile([128, KC, 3 * C], BF16, tag="wqkv")

            def dma_wqkv():
                for kc in range(KC):
                    nc.sync.dma_start(
                        out=wqkv[:, kc, :],
                        in_=wqkvT_h[128 * kc : 128 * (kc + 1), :],
                    )

            wp = constp.tile([128, KC, C], BF16, tag="wp")

            def dma_wp():
                nc.sync.dma_start(
                    out=wp[:], in_=wpT_h[:].rearrange("(k p) o -> p k o", p=128)
                )

            bias_bc = constp.tile([128, C], F32, tag="bias")
            b_src = bias_h[:]
            b_bcast = bass.AP(
                tensor=b_src.tensor, offset=b_src.offset, ap=[[0, 128]] + list(b_src.ap)
            )
            nc.gpsimd.dma_start(out=bias_bc[:], in_=b_bcast)

            ident = constp.tile([128, 128], BF16, tag="ident")
            make_identity(nc, ident[:])
            # mask4[s, j, t] = 1 if t >= s else 0 (causal keep-mask, 4 heads wide)
            mask4 = constp.tile([128, 4, T], BF16, tag="mask4")
            nc.gpsimd.memset(mask4[:], 1.0)
            for j in range(4):
                nc.gpsimd.affine_select(
                    out=mask4[:, j, :],
                    in_=mask4[:, j, :],
                    compare_op=mybir.AluOpType.is_ge,
                    fill=0.0,
                    base=0,
                    pattern=[[1, T]],
                    channel_multiplier=-1,
                )

            # Per-group / per-batch tile handles, created at emission time.
            xt_sb = {}
            qkt_sb = {}
            vaug_sb = {}
            expq_sb = {}
            ot_sb = {}
            qkt_evac_cnt = [0]

            def dma_xt(g):
                xt = xtp.tile([128, KC, GB, T], BF16, tag="xt", name=f"xt{g}")
                xt_sb[g] = xt
                for bi in range(GB):
                    nc.sync.dma_start(out=xt[:, :, bi, :], in_=xT_h[g * GB + bi])

            def emit_qk_chunk(g, r):
                """One 128-row output chunk of the q/k group GEMM: 6 matmuls."""
                if r == 0:
                    qkt_sb[g] = qktp.tile(
                        [128, QK_R, GB, T], BF16, tag="qkt", name=f"qkt{g}"
                    )
                xt = xt_sb[g]
                ps = qkpsp.tile([128, GB, T], F32, tag="qkps", name=f"qkps{g}_{r}")
                for kc in range(KC):
                    yield nc.tensor.matmul(
                        ps[:],
                        lhsT=wqkv[:, kc, 128 * r : 128 * r + 128],
                        rhs=xt[:, kc, :, :],
                        start=(kc == 0),
                        stop=(kc == KC - 1),
                    )
                # alternate evac engine (GpSimd cannot read PSUM)
                if qkt_evac_cnt[0] % 2 == 0:
                    nc.scalar.copy(qkt_sb[g][:, r, :, :], ps[:])
                else:
                    nc.vector.tensor_copy(qkt_sb[g][:, r, :, :], ps[:])
                qkt_evac_cnt[0] += 1

            def emit_v(b):
                """v for batch b into augmented [t, 12, 65] tile (ones col 64)."""
                g, bi = b // GB, b % GB
                vaug = vp.tile([128, H, D + 1], BF16, tag="vaug", name=f"vaug{b}")
                vaug_sb[b] = vaug
                nc.gpsimd.memset(vaug[:, :, D : D + 1], 1.0)
                xt = xt_sb[g]
                for half in range(2):
                    vps = qkpsp.tile(
                        [128, 6, D], F32, tag="qkps", name=f"vps{b}_{half}"
                    )
                    for kc in range(KC):
                        yield nc.tensor.matmul(
                            vps[:],
                            lhsT=xt[:, kc, bi, :],
                            rhs=wqkv[
                                :, kc, 2 * C + 384 * half : 2 * C + 384 * (half + 1)
                            ],
                            start=(kc == 0),
                            stop=(kc == KC - 1),
                        )
                    nc.scalar.copy(vaug[:, 6 * half : 6 * half + 6, 0:D], vps[:])

            def emit_proj(b):
                """proj + bias for batch b (reads ot_sb[b]), then DMA out."""
                ot = ot_sb.pop(b)
                ysb = yp.tile([128, C], F32, tag="ysb", name=f"ysb{b}")
                pj = [None, None]
                for half in range(2):
                    pj[half] = pjpsp.tile(
                        [128, 384], F32, tag="pjps", name=f"pjps{b}_{half}"
                    )
                for kc in range(KC):
                    for half in range(2):
                        yield nc.tensor.matmul(
                            pj[half][:],
                            lhsT=ot[:, kc, :],
                            rhs=wp[:, kc, 384 * half : 384 * (half + 1)],
                            start=(kc == 0),
                            stop=(kc == KC - 1),
                        )
                for half in range(2):
                    nc.vector.tensor_add(
                        ysb[:, 384 * half : 384 * (half + 1)],
                        pj[half][:],
                        bias_bc[:, 384 * half : 384 * (half + 1)],
                    )
                nc.sync.dma_start(out=y_h[b], in_=ysb[:])

            # Head-slot order within a quad: [4q, 4q+2, 4q+1, 4q+3]. The two
            # even heads use qkt partitions 0:64 (po=0), so their S^T
            # matmuls may share one PSUM tile (sliced writes are legal at
            # po=0) and one exp covers both; the odd heads (po=64) must
            # each write a full PSUM tile at offset 0 (HW quirk). The proj
            # weight rows are permuted host-side to match this slot order
            # (see _prep_inputs).
            QUAD_SLOTS = ((0, 0), (1, 2), (2, 1), (3, 3))  # (slot, head_off)

            def emit_sx(b):
                """S^T + exp + causal mask for batch b (12 PE matmuls)."""
                g, bi = b // GB, b % GB
                qkt = qkt_sb[g]
                expq_l = []
                expq_sb[b] = expq_l
                for q4 in range(3):
                    expq = expp.tile(
                        [128, 4, T], BF16, tag="expq", name=f"expq{b}_{q4}"
                    )
                    expq_l.append(expq)
                    sqp = attpsp.tile(
                        [128, 2, T], F32, tag="att", name=f"sqp{b}_{q4}"
                    )
                    for i, h_off in enumerate((0, 2)):
                        ch = (4 * q4 + h_off) // 2
                        yield nc.tensor.matmul(
                            sqp[:, i, :],
                            lhsT=qkt[0:64, KC + ch, bi, :],
                            rhs=qkt[0:64, ch, bi, :],
                            start=True,
                            stop=True,
                        )
                    nc.scalar.activation(
                        out=expq[:, 0:2, :],
                        in_=sqp[:],
                        func=mybir.ActivationFunctionType.Exp,
                        scale=0.125,
                    )
                    for slot, h_off in ((2, 1), (3, 3)):
                        ch = (4 * q4 + h_off) // 2
                        sqj = attpsp.tile(
                            [128, T], F32, tag="att", name=f"sq{b}_{q4}_{slot}"
                        )
                        yield nc.tensor.matmul(
                            sqj[:],
                            lhsT=qkt[64:128, KC + ch, bi, :],
                            rhs=qkt[64:128, ch, bi, :],
                            start=True,
                            stop=True,
                        )
                        nc.scalar.activation(
                            out=expq[:, slot, :],
                            in_=sqj[:],
                            func=mybir.ActivationFunctionType.Exp,
                            scale=0.125,
                        )
                    nc.gpsimd.tensor_mul(expq[:], expq[:], mask4[:])

            def emit_ot(b):
                """O_aug + normalize + transposes for batch b (18 PE
                matmuls); runs one wall-batch after emit_sx(b)."""
                vaug = vaug_sb.pop(b)
                expq_l = expq_sb.pop(b)
                o4 = [None] * 3
                ot = otp.tile([128, KC, T], BF16, tag="ot", name=f"ot{b}")
                ot_sb[b] = ot
                for q4 in range(3):
                    # per-slot stride padded to 72 f32 (288 B) to keep each
                    # matmul's PSUM write base 16B-aligned
                    oa = attpsp.tile(
                        [128, 4, 72], F32, tag="att", name=f"oa{b}_{q4}"
                    )
                    for slot, h_off in QUAD_SLOTS:
                        yield nc.tensor.matmul(
                            oa[:, slot, 0 : D + 1],
                            lhsT=expq_l[q4][:, slot, :],
                            rhs=vaug[:, 4 * q4 + h_off, :],
                            start=True,
                            stop=True,
                        )
                    recip = statsp.tile(
                        [128, 4, 1], F32, tag="recip", name=f"recip{b}_{q4}"
                    )
                    nc.vector.reciprocal(recip[:], oa[:, :, D : D + 1])
                    o4[q4] = o4p.tile([128, 4, D], BF16, tag="o4", name=f"o4{b}_{q4}")
                    # one broadcast multiply normalizes the whole quad:
                    # recip [128, 4, 1] read with stride-0 over d
                    rsrc = recip[:]
                    rbc = bass.AP(
                        tensor=rsrc.tensor,
                        offset=rsrc.offset,
                        ap=[rsrc.ap[0], rsrc.ap[1], [0, D]],
                    )
                    nc.vector.tensor_mul(o4[q4][:], oa[:, :, 0:D], rbc)
                # slot-pair transposes into proj-ready [c, t] chunks (the
                # host-side wp row permutation matches this c ordering)
                for p in range(KC):
                    q4, pj_ = p // 2, p % 2
                    otps = attpsp.tile([128, T], BF16, tag="att", name=f"otps{b}_{p}")
                    yield nc.tensor.transpose(
                        otps[:], o4[q4][:, 2 * pj_ : 2 * pj_ + 2, :], ident[:]
                    )
                    nc.vector.tensor_copy(ot[:, p, :], otps[:])

            # ---------------- prologue ----------------
            dma_xt(0)
            dma_wqkv()
            if n_groups > 1:
                dma_xt(1)
            dma_wp()
            for r in range(QK_R):
                for _ in emit_qk_chunk(0, r):
                    pass
            for _ in emit_v(0):
                pass

            # ---------------- main loop ----------------
            # 3-deep software pipeline: wall-iteration b emits S^T(b),
            # O/transpose(b-1), proj(b-2), plus v(b+1) and qk chunks of
            # group g+1 as long fillers.
            def merge(att_gens, n_att, fill_gens, n_fill):
                # Small matmuls are emitted in PAIRS between long fillers:
                # the second small's (cheap) weight-load hides under the
                # first small's execution, so only one long-LDWEIGHTS
                # penalty is paid per pair instead of two.
                att_iter = itertools.chain(*att_gens)
                fill_iter = itertools.chain(*fill_gens)
                n_ap = (n_att + 1) // 2
                ia = if_ = 0
                while ia < n_ap or if_ < n_fill:
                    if if_ < n_fill and (ia >= n_ap or if_ * n_ap <= ia * n_fill):
                        next(fill_iter, None)
                        if_ += 1
                    else:
                        next(att_iter, None)
                        next(att_iter, None)
                        ia += 1
                for _ in att_iter:
                    pass
                for _ in fill_iter:
                    pass

            for b in range(b_core):
                g, bi = b // GB, b % GB
                # prefetch xt two groups ahead (buffer freed mid-group)
                if bi == 3 and g + 2 < n_groups:
                    dma_xt(g + 2)

                att_gens = []
                n_att = 12
                if b >= 1:
                    att_gens.append(emit_ot(b - 1))
                    n_att += 18
                att_gens.append(emit_sx(b))

                fill_gens = []
                n_fill = 0
                if g + 1 < n_groups:
                    r0 = sum(QK_SPLIT[:bi])
                    for r in range(r0, r0 + QK_SPLIT[bi]):
                        fill_gens.append(emit_qk_chunk(g + 1, r))
                        n_fill += 6
                if b + 1 < b_core:
                    fill_gens.append(emit_v(b + 1))
                    n_fill += 12
                if b >= 2:
                    fill_gens.append(emit_proj(b - 2))
                    n_fill += 12
                if b == b_core - 1:
                    # pull proj(b-1) into the last iteration: ot(b-1) runs
                    # at the head of this iteration's small stream, so its
                    # transposes are evac'd by the time these late fills run
                    fill_gens.append(emit_proj(b - 1))
                    n_fill += 12

                merge(att_gens, n_att, fill_gens, n_fill)

            # ---------------- epilogue ----------------
            for _ in emit_ot(b_core - 1):
                pass
            for _ in emit_proj(b_core - 1):
                pass

    nc.finalize()
    return nc


_NC_CACHE = {}


def _get_nc(b_core=B_CORE):
    if b_core not in _NC_CACHE:
        _NC_CACHE[b_core] = _build(b_core)
    return _NC_CACHE[b_core]


def _prep_inputs(x, w_qkv, w_proj, b_proj, b_core):
    import ml_dtypes

    x = np.asarray(x, dtype=np.float32)
    n_cores = x.shape[0] // b_core
    # [B, T, C] -> [B, C, T] -> [B, KC, 128, T] -> [B, 128, KC, T]
    xT = (
        np.ascontiguousarray(
            x.transpose(0, 2, 1)
            .reshape(x.shape[0], KC, 128, T)
            .transpose(0, 2, 1, 3)
        )
        .astype(ml_dtypes.bfloat16)
    )
    wqkvT = np.ascontiguousarray(np.asarray(w_qkv, dtype=np.float32).T).astype(
        ml_dtypes.bfloat16
    )
    # The kernel emits attention heads in per-quad slot order
    # [4q, 4q+2, 4q+1, 4q+3], so the proj-input channel order of its ot
    # chunks is permuted; permute the wp rows to match.
    perm = []
    for q4 in range(H // 4):
        for h_off in (0, 2, 1, 3):
            h = 4 * q4 + h_off
            perm.extend(range(D * h, D * h + D))
    wpT = np.ascontiguousarray(
        np.asarray(w_proj, dtype=np.float32).T[perm, :]
    ).astype(ml_dtypes.bfloat16)
    bias = np.ascontiguousarray(np.asarray(b_proj, dtype=np.float32))
    in_maps = []
    for c in range(n_cores):
        in_maps.append(
            {
                "xT": np.ascontiguousarray(xT[c * b_core : (c + 1) * b_core]),
                "wqkvT": wqkvT,
                "wpT": wpT,
                "bias": bias,
            }
        )
    return in_maps


def run(x, w_qkv, w_proj, b_proj, b_core=B_CORE, trace=False):
    nc = _get_nc(b_core)
    n_cores = x.shape[0] // b_core
    in_maps = _prep_inputs(x, w_qkv, w_proj, b_proj, b_core)
    res = run_bass_kernel_spmd(nc, in_maps, list(range(n_cores)), trace=trace)
    y = np.concatenate([res.results[i]["y"] for i in range(n_cores)], axis=0)
    return y, res


def kernel(x, w_qkv, w_proj, b_proj):
    y, _ = run(x, w_qkv, w_proj, b_proj)
    return y
